# revision 14
# baseline (speedup 1.0000x reference)
"""Trainium2 Bass kernel for DynamicLocalGlobalRouter.

Reference computation (B=2, H=16, S=2048, D=64, radius=16):
  local_out = sliding-window softmax attention (window 33) per (b,h)
  gate      = sigmoid(mean_s(q) @ w_gate + b_gate)      per (b,h)
  out       = gate * local_out + (1-gate) * global_out

Sharding: B*H = 32 (b,h) pairs -> 4 pairs per core across 8 cores.

Device algorithm (per pair), key-stationary banded attention:
  - Host supplies q/k transposed+padded to [64, S+pad] so the contraction
    dim (d=64) is on partitions; no on-device transposes are needed.
  - For each key chunk c (17 chunks of 128 keys, shifted by -16):
      scores_T[k, q] = K_T_chunk.T @ Q_T_span        (PE, psum [128,160])
      P_T = exp(scores_T)                            (ACT; no max-subtract:
                                                      scores are O(1) randn)
      P_T *= band_mask                               (DVE, zeroes out-of-band)
      pv[block] += P_T_cols.T @ [V | 1/g]            (PE; extra column gives
                                                      Z/g = sum of weights)
  - evict: out_block = pv[:, 0:64] * (g/Z)           (ACT copy w/ scale AP)
  - blend: out += (1-g) * global_out                 (DVE)
The softmax normalization (1/Z), gate, and boundary masking are exact:
zero-padded K columns give exp(0)=1 which the band mask multiplies to 0,
and out-of-range V rows/ones-column entries are zero.
"""

import os
import sys
from contextlib import ExitStack

import numpy as np

sys.path.insert(0, "/opt/trn_rl_repo")

import concourse.bacc as bacc  # noqa: E402
import concourse.tile as tile  # noqa: E402
from concourse import mybir  # noqa: E402
from concourse.bass_utils import run_bass_kernel_spmd  # noqa: E402

B, H, S, D = 2, 16, 2048, 64
RADIUS = 16
NCORES = 8
PAIRS = B * H            # 32
PPC = PAIRS // NCORES    # 4 pairs per core
NB = S // 128            # 16 query/key blocks
NCH = NB + 1             # 17 key chunks (chunk c covers keys [c*128-16, c*128+112))
SPAN = 160               # query span per key chunk
QT_W = 32 + S + 128      # 2208 padded Q_T width (col i <-> query i-32)
KT_W = 16 + S + 112      # 2176 padded K_T width (col j <-> key j-16)
VS_W = NCH * 65          # V chunks with appended ones/invg column

F32 = mybir.dt.float32

# set by test harness to capture an NTFF profile
TRACE = bool(int(os.environ.get("KERNEL_TRACE", "0")))
LAST_RESULT = None

_CACHE = {}


def _build_program(nc, reps=None):
    qt_d = nc.dram_tensor("qt", [PPC, 64, QT_W], F32, kind="ExternalInput")
    kt_d = nc.dram_tensor("kt", [PPC, 64, KT_W], F32, kind="ExternalInput")
    vs_d = nc.dram_tensor("vs", [PPC, 128, VS_W], F32, kind="ExternalInput")
    gl_d = nc.dram_tensor("gl", [PPC, 128, NB * 64], F32, kind="ExternalInput")
    mask_d = nc.dram_tensor("mask", [128, SPAN], F32, kind="ExternalInput")
    wg_d = nc.dram_tensor("wg", [64, 1], F32, kind="ExternalInput")
    bg_d = nc.dram_tensor("bg", [1, 1], F32, kind="ExternalInput")
    out_d = nc.dram_tensor("out", [PPC, 128, NB * 64], F32, kind="ExternalOutput")

    with tile.TileContext(nc) as tc, ExitStack() as ctx:
        consts = ctx.enter_context(tc.tile_pool(name="consts", bufs=1))
        pairp = ctx.enter_context(tc.tile_pool(name="pairp", bufs=2))
        smalls = ctx.enter_context(tc.tile_pool(name="smalls", bufs=2))
        ppool = ctx.enter_context(tc.tile_pool(name="ppool", bufs=3))
        zpool = ctx.enter_context(tc.tile_pool(name="zpool", bufs=4))
        ps_s = ctx.enter_context(tc.tile_pool(name="ps_s", bufs=2, space="PSUM"))
        ps_pv = ctx.enter_context(tc.tile_pool(name="ps_pv", bufs=4, space="PSUM"))
        ps_g = ctx.enter_context(tc.tile_pool(name="ps_g", bufs=1, space="PSUM"))

        mask_sb = consts.tile([128, SPAN], F32, tag="mask")
        nc.sync.dma_start(out=mask_sb, in_=mask_d[:, :])
        wg_sb = consts.tile([64, 1], F32, tag="wg")
        nc.sync.dma_start(out=wg_sb, in_=wg_d[:, :])
        bg_sb = consts.tile([1, 1], F32, tag="bg")
        nc.sync.dma_start(out=bg_sb, in_=bg_d[:, :])
        ones_sb = consts.tile([1, 128], F32, tag="ones")
        nc.vector.memset(ones_sb, 1.0)

        def emit_pair(p):
            qt = pairp.tile([64, QT_W], F32, tag="qt")
            nc.sync.dma_start(out=qt, in_=qt_d[p])
            kt = pairp.tile([64, KT_W], F32, tag="kt")
            nc.sync.dma_start(out=kt, in_=kt_d[p])
            vs = pairp.tile([128, VS_W], F32, tag="vs")
            nc.sync.dma_start(out=vs, in_=vs_d[p])
            vs3 = vs.rearrange("p (c w) -> p c w", w=65)
            gl = pairp.tile([128, NB * 64], F32, tag="gl")
            nc.sync.dma_start(out=gl, in_=gl_d[p])
            outp = pairp.tile([128, NB * 64], F32, tag="outp")

            # ---- gate: g = sigmoid(mean_s(q) . w + b) ----
            g_ps = ps_g.tile([1, 128], F32, tag="gps")
            for t in range(NB):
                nc.tensor.matmul(
                    g_ps,
                    lhsT=wg_sb,
                    rhs=qt[:, 32 + t * 128 : 32 + (t + 1) * 128],
                    start=(t == 0),
                    stop=(t == NB - 1),
                )
            # scl2 cols: 0 = sum(q.w), 1 = gate, 2 = 1-gate, 3 = 1/gate
            scl2 = smalls.tile([1, 4], F32, tag="scl2")
            nc.vector.reduce_sum(scl2[:, 0:1], g_ps, axis=mybir.AxisListType.X)
            nc.scalar.activation(
                scl2[:, 1:2],
                scl2[:, 0:1],
                mybir.ActivationFunctionType.Sigmoid,
                bias=bg_sb[0:1, 0:1],
                scale=1.0 / S,
            )
            nc.vector.reciprocal(scl2[:, 3:4], scl2[:, 1:2])
            nc.vector.tensor_scalar(
                scl2[:, 2:3],
                scl2[:, 1:2],
                -1.0,
                1.0,
                op0=mybir.AluOpType.mult,
                op1=mybir.AluOpType.add,
            )
            # broadcast (1-g, 1/g) across 128 partitions via ones matmul
            bc_ps = ps_s.tile([128, 2], F32, tag="st")
            nc.tensor.matmul(bc_ps, lhsT=ones_sb, rhs=scl2[:, 2:4], start=True, stop=True)
            bc = smalls.tile([128, 2], F32, tag="bc")
            nc.vector.tensor_copy(bc, bc_ps)
            # scale the appended V column (1 for valid keys) by 1/g so the
            # Z-column of pv comes out as Z/g and the evict scale is g/Z.
            nc.vector.tensor_scalar_mul(vs3[:, :, 64:65], vs3[:, :, 64:65], bc[:, 1:2])

            # ---- banded attention over 17 key chunks ----
            pvs = [None] * NB
            for c in range(NCH):
                st = ps_s.tile([128, SPAN], F32, tag="st")
                nc.tensor.matmul(
                    st,
                    lhsT=kt[:, c * 128 : (c + 1) * 128],
                    rhs=qt[:, c * 128 : c * 128 + SPAN],
                    start=True,
                    stop=True,
                )
                pT = ppool.tile([128, SPAN], F32, tag="pT")
                # exp(scores / sqrt(D)) -- the 1/8 score scale rides the
                # activation's scale input
                nc.scalar.activation(
                    pT, st, mybir.ActivationFunctionType.Exp, scale=1.0 / np.sqrt(D)
                )
                nc.vector.tensor_mul(pT, pT, mask_sb)
                if c < NB:
                    pv = ps_pv.tile([128, 65], F32, tag="pv")
                    pvs[c] = pv
                    nc.tensor.matmul(
                        pv,
                        lhsT=pT[:, 32:SPAN],
                        rhs=vs3[:, c, :],
                        start=True,
                        stop=False,
                        skip_group_check=True,
                    )
                if c > 0:
                    b = c - 1
                    nc.tensor.matmul(
                        pvs[b][96:128, :],
                        lhsT=pT[:, 0:32],
                        rhs=vs3[:, c, :],
                        start=False,
                        stop=True,
                        skip_group_check=True,
                        tile_position=(0, 96),
                    )
                    zr = zpool.tile([128, 1], F32, tag="zr")
                    nc.vector.reciprocal(zr, pvs[b][:, 64:65])
                    nc.scalar.activation(
                        outp[:, b * 64 : (b + 1) * 64],
                        pvs[b][:, 0:64],
                        mybir.ActivationFunctionType.Copy,
                        bias=0.0,
                        scale=zr,
                    )

            # ---- blend with global path ----
            nc.vector.tensor_scalar_mul(gl, gl, bc[:, 0:1])
            nc.vector.tensor_add(outp, outp, gl)
            nc.sync.dma_start(out=out_d[p], in_=outp)

        if reps is None:
            for p in range(PPC):
                emit_pair(p)
        else:
            # benchmark variant: repeat the whole body in-NEFF so wall-clock
            # deltas between rep counts measure pure HW iteration time
            with tc.For_i(0, reps, 1):
                for p in range(PPC):
                    emit_pair(p)


def _get_nc(reps=None):
    key = ("nc", reps)
    if key not in _CACHE:
        nc = bacc.Bacc("TRN2", target_bir_lowering=False)
        _build_program(nc, reps=reps)
        nc.compile()
        _CACHE[key] = nc
    return _CACHE[key]


def _band_mask():
    j = np.arange(128)[:, None]
    i = np.arange(SPAN)[None, :]
    return ((j <= i) & (j >= i - 32)).astype(np.float32)


def _prepare_in_maps(inputs):
    q = np.ascontiguousarray(np.asarray(inputs["q"], dtype=np.float32))
    k = np.ascontiguousarray(np.asarray(inputs["k"], dtype=np.float32))
    v = np.ascontiguousarray(np.asarray(inputs["v"], dtype=np.float32))
    g = np.ascontiguousarray(np.asarray(inputs["global_out"], dtype=np.float32))
    wg = np.asarray(inputs["w_gate"], dtype=np.float32).reshape(64, 1)
    bg = np.asarray(inputs["b_gate"], dtype=np.float32).reshape(1, 1)

    qf = q.reshape(PAIRS, S, D)
    kf = k.reshape(PAIRS, S, D)
    vf = v.reshape(PAIRS, S, D)
    gf = g.reshape(PAIRS, S, D)

    # host-side layout marshalling (transpose/pad/shift only, no math)
    qt = np.zeros((PAIRS, 64, QT_W), np.float32)
    qt[:, :, 32 : 32 + S] = qf.transpose(0, 2, 1)
    kt = np.zeros((PAIRS, 64, KT_W), np.float32)
    kt[:, :, 16 : 16 + S] = kf.transpose(0, 2, 1)

    vs = np.zeros((PAIRS, NCH * 128, 65), np.float32)
    vs[:, 16 : 16 + S, 0:64] = vf
    vs[:, 16 : 16 + S, 64] = 1.0
    vs = (
        vs.reshape(PAIRS, NCH, 128, 65)
        .transpose(0, 2, 1, 3)
        .reshape(PAIRS, 128, VS_W)
    )
    vs = np.ascontiguousarray(vs)

    gl = np.ascontiguousarray(
        gf.reshape(PAIRS, NB, 128, 64).transpose(0, 2, 1, 3).reshape(PAIRS, 128, NB * 64)
    )
    mask = _band_mask()

    in_maps = []
    for core in range(NCORES):
        lo, hi = core * PPC, (core + 1) * PPC
        in_maps.append(
            {
                "qt": np.ascontiguousarray(qt[lo:hi]),
                "kt": np.ascontiguousarray(kt[lo:hi]),
                "vs": vs[lo:hi],
                "gl": gl[lo:hi],
                "mask": mask,
                "wg": wg,
                "bg": bg,
            }
        )
    return in_maps


def kernel(**inputs):
    global LAST_RESULT
    in_maps = _prepare_in_maps(inputs)
    nc = _get_nc()
    try:
        res = run_bass_kernel_spmd(
            nc, in_maps, core_ids=list(range(NCORES)), trace=TRACE
        )
    except ModuleNotFoundError:
        # NTFF profiling hook unavailable in this axon build
        res = run_bass_kernel_spmd(
            nc, in_maps, core_ids=list(range(NCORES)), trace=False
        )
    LAST_RESULT = res

    outs = np.stack([res.results[i]["out"] for i in range(NCORES)])  # [8,4,128,NB*64]
    out = (
        outs.reshape(PAIRS, 128, NB, 64)
        .transpose(0, 2, 1, 3)
        .reshape(B, H, S, D)
    )
    return np.ascontiguousarray(out)


def bench_hw_ns(inputs, reps_lo=16, reps_hi=2064, runs=5):
    """Estimate per-invocation HW time via in-NEFF repetition.

    Runs the same program with the body looped reps_lo and reps_hi times;
    the wall-clock delta divided by the rep delta isolates on-device time
    from compile/shipping/dispatch overhead.
    """
    import time

    in_maps = _prepare_in_maps(inputs)

    def run_variant(reps):
        nc = _get_nc(reps=reps)
        times = []
        for r in range(runs + 1):
            t0 = time.time()
            run_bass_kernel_spmd(nc, in_maps, core_ids=list(range(NCORES)))
            t1 = time.time()
            if r > 0:  # first run includes NEFF compile
                times.append(t1 - t0)
        return min(times)

    t_lo = run_variant(reps_lo)
    t_hi = run_variant(reps_hi)
    per_iter_ns = (t_hi - t_lo) / (reps_hi - reps_lo) * 1e9
    return per_iter_ns, t_lo, t_hi


if __name__ == "__main__":
    rng = np.random.default_rng(0)
    ins = {
        "q": rng.standard_normal((B, H, S, D), dtype=np.float32),
        "k": rng.standard_normal((B, H, S, D), dtype=np.float32),
        "v": rng.standard_normal((B, H, S, D), dtype=np.float32),
        "global_out": rng.standard_normal((B, H, S, D), dtype=np.float32),
        "buckets": rng.integers(0, 64, size=(B, S)),
        "w_gate": rng.standard_normal(64, dtype=np.float32) / 8.0,
        "b_gate": np.zeros(1, np.float32),
    }
    out = kernel(**ins)
    print("out", out.shape, out.dtype, float(np.abs(out).max()))


# revision 21
# speedup vs baseline: 1.4319x; 1.4319x over previous
"""Trainium2 Bass kernel for DynamicLocalGlobalRouter.

Reference computation (B=2, H=16, S=2048, D=64, radius=16):
  local_out = sliding-window softmax attention (window 33) per (b,h)
  gate      = sigmoid(mean_s(q) @ w_gate + b_gate)      per (b,h)
  out       = gate * local_out + (1-gate) * global_out

Sharding: B*H = 32 (b,h) pairs -> 4 pairs per core across 8 cores.

Device algorithm (per pair), key-stationary banded attention:
  - Host supplies q/k transposed+padded to [64, S+pad] so the contraction
    dim (d=64) is on partitions; no on-device transposes are needed.
  - For each key chunk c (17 chunks of 128 keys, shifted by -16):
      scores_T[k, q] = K_T_chunk.T @ Q_T_span        (PE, psum [128,160])
      P_T = exp(scores_T)                            (ACT; no max-subtract:
                                                      scores are O(1) randn)
      P_T *= band_mask                               (DVE, zeroes out-of-band)
      pv[block] += P_T_cols.T @ [V | 1/g]            (PE; extra column gives
                                                      Z/g = sum of weights)
  - evict: out_block = pv[:, 0:64] * (g/Z)           (ACT copy w/ scale AP)
  - blend: out += (1-g) * global_out                 (DVE)
The softmax normalization (1/Z), gate, and boundary masking are exact:
zero-padded K columns give exp(0)=1 which the band mask multiplies to 0,
and out-of-range V rows/ones-column entries are zero.
"""

import os
import sys
from contextlib import ExitStack

import numpy as np

sys.path.insert(0, "/opt/trn_rl_repo")

import concourse.bacc as bacc  # noqa: E402
import concourse.tile as tile  # noqa: E402
from concourse import mybir  # noqa: E402
from concourse.bass_utils import run_bass_kernel_spmd  # noqa: E402

B, H, S, D = 2, 16, 2048, 64
RADIUS = 16
NCORES = 8
PAIRS = B * H            # 32
PPC = PAIRS // NCORES    # 4 pairs per core
NB = S // 128            # 16 query/key blocks
NCH = NB + 1             # 17 key chunks (chunk c covers keys [c*128-16, c*128+112))
SPAN = 160               # query span per key chunk
QT_W = 32 + S + 128      # 2208 padded Q_T width (col i <-> query i-32)
KT_W = 16 + S + 112      # 2176 padded K_T width (col j <-> key j-16)
VS_W = NCH * 65          # V chunks with appended ones/invg column

F32 = mybir.dt.float32

# set by test harness to capture an NTFF profile
TRACE = bool(int(os.environ.get("KERNEL_TRACE", "0")))
LAST_RESULT = None

_CACHE = {}


def _build_program(nc, reps=None):
    # qt/kt stack two pairs on the partition axis (pair 2j on partitions
    # 0:64, pair 2j+1 on 64:128) so their DMAs use all 16 ports.
    qt_d = nc.dram_tensor("qt", [PPC // 2, 128, QT_W], F32, kind="ExternalInput")
    kt_d = nc.dram_tensor("kt", [PPC // 2, 128, KT_W], F32, kind="ExternalInput")
    vs_d = nc.dram_tensor("vs", [PPC, 128, VS_W], F32, kind="ExternalInput")
    gl_d = nc.dram_tensor("gl", [PPC, 128, NB * 64], F32, kind="ExternalInput")
    mask_d = nc.dram_tensor("mask", [128, SPAN], F32, kind="ExternalInput")
    wg_d = nc.dram_tensor("wg", [128, 1], F32, kind="ExternalInput")
    bgn_d = nc.dram_tensor("bgn", [1, 1], F32, kind="ExternalInput")
    out_d = nc.dram_tensor("out", [PPC, 128, NB * 64], F32, kind="ExternalOutput")

    with tile.TileContext(nc) as tc, ExitStack() as ctx:
        consts = ctx.enter_context(tc.tile_pool(name="consts", bufs=1))
        pairp = ctx.enter_context(tc.tile_pool(name="pairp", bufs=2))
        smalls = ctx.enter_context(tc.tile_pool(name="smalls", bufs=2))
        ppool = ctx.enter_context(tc.tile_pool(name="ppool", bufs=3))
        zpool = ctx.enter_context(tc.tile_pool(name="zpool", bufs=4))
        ps_s = ctx.enter_context(tc.tile_pool(name="ps_s", bufs=2, space="PSUM"))
        ps_pv = ctx.enter_context(tc.tile_pool(name="ps_pv", bufs=4, space="PSUM"))
        ps_g = ctx.enter_context(tc.tile_pool(name="ps_g", bufs=1, space="PSUM"))

        mask_sb = consts.tile([128, SPAN], F32, tag="mask")
        nc.sync.dma_start(out=mask_sb, in_=mask_d[:, :])
        wg_sb = consts.tile([128, 1], F32, tag="wg")
        nc.sync.dma_start(out=wg_sb, in_=wg_d[:, :])
        bgn_sb = consts.tile([1, 1], F32, tag="bgn")
        nc.sync.dma_start(out=bgn_sb, in_=bgn_d[:, :])
        ones_sb = consts.tile([1, 128], F32, tag="ones")
        nc.vector.memset(ones_sb, 1.0)

        def emit_pair(p, qt2, kt2):
            b0 = (p % 2) * 64
            qt = qt2[b0 : b0 + 64, :]
            kt = kt2[b0 : b0 + 64, :]
            vs = pairp.tile([128, VS_W], F32, tag="vs")
            nc.sync.dma_start(out=vs, in_=vs_d[p])
            vs3 = vs.rearrange("p (c w) -> p c w", w=65)
            gl = pairp.tile([128, NB * 64], F32, tag="gl")
            nc.sync.dma_start(out=gl, in_=gl_d[p])
            outp = pairp.tile([128, NB * 64], F32, tag="outp")

            # ---- gate: g = sigmoid(mean_s(q) . w + b), via exp only so the
            # ACT engine never has to swap activation tables ----
            g_ps = ps_g.tile([1, 128], F32, tag="gps")
            for t in range(NB):
                nc.tensor.matmul(
                    g_ps,
                    lhsT=wg_sb[b0 : b0 + 64, :],
                    rhs=qt[:, 32 + t * 128 : 32 + (t + 1) * 128],
                    start=(t == 0),
                    stop=(t == NB - 1),
                )
            # scl2 cols: 0 = sum(q.w), 3 = g, 4 = 1-g, 5 = 1/g = 1+exp(-x)
            scl2 = smalls.tile([1, 6], F32, tag="scl2")
            nc.vector.reduce_sum(scl2[:, 0:1], g_ps, axis=mybir.AxisListType.X)
            nc.scalar.activation(
                scl2[:, 5:6],
                scl2[:, 0:1],
                mybir.ActivationFunctionType.Exp,
                bias=bgn_sb[0:1, 0:1],
                scale=-1.0 / S,
            )
            nc.vector.tensor_scalar(
                scl2[:, 5:6], scl2[:, 5:6], 1.0, None, op0=mybir.AluOpType.add
            )
            nc.vector.reciprocal(scl2[:, 3:4], scl2[:, 5:6])
            nc.vector.tensor_scalar(
                scl2[:, 4:5],
                scl2[:, 3:4],
                -1.0,
                1.0,
                op0=mybir.AluOpType.mult,
                op1=mybir.AluOpType.add,
            )
            # broadcast (1-g, 1/g) across 128 partitions via ones matmul
            bc_ps = ps_s.tile([128, 2], F32, tag="st")
            nc.tensor.matmul(bc_ps, lhsT=ones_sb, rhs=scl2[:, 4:6], start=True, stop=True)
            bc = smalls.tile([128, 2], F32, tag="bc")
            nc.vector.tensor_copy(bc, bc_ps)
            # scale the appended V column (1 for valid keys) by 1/g so the
            # Z-column of pv comes out as Z/g and the evict scale is g/Z.
            nc.vector.tensor_scalar_mul(vs3[:, :, 64:65], vs3[:, :, 64:65], bc[:, 1:2])

            # ---- banded attention over 17 key chunks ----
            pvs = [None] * NB
            for c in range(NCH):
                st = ps_s.tile([128, SPAN], F32, tag="st")
                nc.tensor.matmul(
                    st,
                    lhsT=kt[:, c * 128 : (c + 1) * 128],
                    rhs=qt[:, c * 128 : c * 128 + SPAN],
                    start=True,
                    stop=True,
                )
                pT = ppool.tile([128, SPAN], F32, tag="pT")
                # exp(scores / sqrt(D)) -- the 1/8 score scale rides the
                # activation's scale input
                nc.scalar.activation(
                    pT, st, mybir.ActivationFunctionType.Exp, scale=1.0 / np.sqrt(D)
                )
                nc.vector.tensor_mul(pT, pT, mask_sb)
                if c < NB:
                    pv = ps_pv.tile([128, 65], F32, tag="pv")
                    pvs[c] = pv
                    nc.tensor.matmul(
                        pv,
                        lhsT=pT[:, 32:SPAN],
                        rhs=vs3[:, c, :],
                        start=True,
                        stop=False,
                        skip_group_check=True,
                    )
                if c > 0:
                    b = c - 1
                    nc.tensor.matmul(
                        pvs[b][96:128, :],
                        lhsT=pT[:, 0:32],
                        rhs=vs3[:, c, :],
                        start=False,
                        stop=True,
                        skip_group_check=True,
                        tile_position=(0, 96),
                    )
                    zr = zpool.tile([128, 1], F32, tag="zr")
                    nc.vector.reciprocal(zr, pvs[b][:, 64:65])
                    nc.scalar.activation(
                        outp[:, b * 64 : (b + 1) * 64],
                        pvs[b][:, 0:64],
                        mybir.ActivationFunctionType.Copy,
                        bias=0.0,
                        scale=zr,
                    )

            # ---- blend with global path ----
            nc.vector.tensor_scalar_mul(gl, gl, bc[:, 0:1])
            nc.vector.tensor_add(outp, outp, gl)
            nc.sync.dma_start(out=out_d[p], in_=outp)

        def emit_all():
            for grp in range(PPC // 2):
                qt2 = pairp.tile([128, QT_W], F32, tag="qt")
                nc.sync.dma_start(out=qt2, in_=qt_d[grp])
                kt2 = pairp.tile([128, KT_W], F32, tag="kt")
                nc.sync.dma_start(out=kt2, in_=kt_d[grp])
                for sub in range(2):
                    emit_pair(grp * 2 + sub, qt2, kt2)

        if reps is None:
            emit_all()
        else:
            # benchmark variant: repeat the whole body in-NEFF so wall-clock
            # deltas between rep counts measure pure HW iteration time
            engs = [
                mybir.EngineType.PE,
                mybir.EngineType.Activation,
                mybir.EngineType.DVE,
                mybir.EngineType.Pool,
                mybir.EngineType.SP,
            ]
            with tc.For_i(0, reps, 1, hint_engines=engs):
                emit_all()


def _get_nc(reps=None):
    key = ("nc", reps)
    if key not in _CACHE:
        nc = bacc.Bacc("TRN2", target_bir_lowering=False)
        _build_program(nc, reps=reps)
        nc.compile()
        _CACHE[key] = nc
    return _CACHE[key]


def _band_mask():
    j = np.arange(128)[:, None]
    i = np.arange(SPAN)[None, :]
    return ((j <= i) & (j >= i - 32)).astype(np.float32)


def _prepare_in_maps(inputs):
    q = np.ascontiguousarray(np.asarray(inputs["q"], dtype=np.float32))
    k = np.ascontiguousarray(np.asarray(inputs["k"], dtype=np.float32))
    v = np.ascontiguousarray(np.asarray(inputs["v"], dtype=np.float32))
    g = np.ascontiguousarray(np.asarray(inputs["global_out"], dtype=np.float32))
    wg = np.asarray(inputs["w_gate"], dtype=np.float32).reshape(64, 1)
    wg = np.ascontiguousarray(np.concatenate([wg, wg], axis=0))  # [128,1]
    bgn = -np.asarray(inputs["b_gate"], dtype=np.float32).reshape(1, 1)

    qf = q.reshape(PAIRS, S, D)
    kf = k.reshape(PAIRS, S, D)
    vf = v.reshape(PAIRS, S, D)
    gf = g.reshape(PAIRS, S, D)

    # host-side layout marshalling (transpose/pad/shift only, no math);
    # qt/kt stack pair 2j on partitions 0:64 and pair 2j+1 on 64:128
    qt = np.zeros((PAIRS // 2, 128, QT_W), np.float32)
    qt[:, 0:64, 32 : 32 + S] = qf[0::2].transpose(0, 2, 1)
    qt[:, 64:128, 32 : 32 + S] = qf[1::2].transpose(0, 2, 1)
    kt = np.zeros((PAIRS // 2, 128, KT_W), np.float32)
    kt[:, 0:64, 16 : 16 + S] = kf[0::2].transpose(0, 2, 1)
    kt[:, 64:128, 16 : 16 + S] = kf[1::2].transpose(0, 2, 1)

    vs = np.zeros((PAIRS, NCH * 128, 65), np.float32)
    vs[:, 16 : 16 + S, 0:64] = vf
    vs[:, 16 : 16 + S, 64] = 1.0
    vs = (
        vs.reshape(PAIRS, NCH, 128, 65)
        .transpose(0, 2, 1, 3)
        .reshape(PAIRS, 128, VS_W)
    )
    vs = np.ascontiguousarray(vs)

    gl = np.ascontiguousarray(
        gf.reshape(PAIRS, NB, 128, 64).transpose(0, 2, 1, 3).reshape(PAIRS, 128, NB * 64)
    )
    mask = _band_mask()

    in_maps = []
    for core in range(NCORES):
        lo, hi = core * PPC, (core + 1) * PPC
        glo, ghi = core * (PPC // 2), (core + 1) * (PPC // 2)
        in_maps.append(
            {
                "qt": np.ascontiguousarray(qt[glo:ghi]),
                "kt": np.ascontiguousarray(kt[glo:ghi]),
                "vs": vs[lo:hi],
                "gl": gl[lo:hi],
                "mask": mask,
                "wg": wg,
                "bgn": bgn,
            }
        )
    return in_maps


def kernel(**inputs):
    global LAST_RESULT
    in_maps = _prepare_in_maps(inputs)
    nc = _get_nc()
    try:
        res = run_bass_kernel_spmd(
            nc, in_maps, core_ids=list(range(NCORES)), trace=TRACE
        )
    except ModuleNotFoundError:
        # NTFF profiling hook unavailable in this axon build
        res = run_bass_kernel_spmd(
            nc, in_maps, core_ids=list(range(NCORES)), trace=False
        )
    LAST_RESULT = res

    outs = np.stack([res.results[i]["out"] for i in range(NCORES)])  # [8,4,128,NB*64]
    out = (
        outs.reshape(PAIRS, 128, NB, 64)
        .transpose(0, 2, 1, 3)
        .reshape(B, H, S, D)
    )
    return np.ascontiguousarray(out)


def bench_hw_ns(inputs, reps_lo=16, reps_hi=2064, runs=5):
    """Estimate per-invocation HW time via in-NEFF repetition.

    Runs the same program with the body looped reps_lo and reps_hi times;
    the wall-clock delta divided by the rep delta isolates on-device time
    from compile/shipping/dispatch overhead.
    """
    import time

    in_maps = _prepare_in_maps(inputs)

    def run_variant(reps):
        nc = _get_nc(reps=reps)
        times = []
        for r in range(runs + 1):
            t0 = time.time()
            run_bass_kernel_spmd(nc, in_maps, core_ids=list(range(NCORES)))
            t1 = time.time()
            if r > 0:  # first run includes NEFF compile
                times.append(t1 - t0)
        return min(times)

    t_lo = run_variant(reps_lo)
    t_hi = run_variant(reps_hi)
    per_iter_ns = (t_hi - t_lo) / (reps_hi - reps_lo) * 1e9
    return per_iter_ns, t_lo, t_hi


if __name__ == "__main__":
    rng = np.random.default_rng(0)
    ins = {
        "q": rng.standard_normal((B, H, S, D), dtype=np.float32),
        "k": rng.standard_normal((B, H, S, D), dtype=np.float32),
        "v": rng.standard_normal((B, H, S, D), dtype=np.float32),
        "global_out": rng.standard_normal((B, H, S, D), dtype=np.float32),
        "buckets": rng.integers(0, 64, size=(B, S)),
        "w_gate": rng.standard_normal(64, dtype=np.float32) / 8.0,
        "b_gate": np.zeros(1, np.float32),
    }
    out = kernel(**ins)
    print("out", out.shape, out.dtype, float(np.abs(out).max()))


# revision 29
# speedup vs baseline: 1.5671x; 1.0944x over previous
"""Trainium2 Bass kernel for DynamicLocalGlobalRouter.

Reference computation (B=2, H=16, S=2048, D=64, radius=16):
  local_out = sliding-window softmax attention (window 33) per (b,h)
  gate      = sigmoid(mean_s(q) @ w_gate + b_gate)      per (b,h)
  out       = gate * local_out + (1-gate) * global_out

Sharding: B*H = 32 (b,h) pairs -> 4 pairs per core across 8 cores.

Device algorithm (per pair), key-stationary banded attention:
  - Host supplies q/k transposed+padded to [64, S+pad] so the contraction
    dim (d=64) is on partitions; no on-device transposes are needed.
  - For each key chunk c (17 chunks of 128 keys, shifted by -16):
      scores_T[k, q] = K_T_chunk.T @ Q_T_span        (PE, psum [128,160])
      P_T = exp(scores_T)                            (ACT; no max-subtract:
                                                      scores are O(1) randn)
      P_T *= band_mask                               (DVE, zeroes out-of-band)
      pv[block] += P_T_cols.T @ [V | 1/g]            (PE; extra column gives
                                                      Z/g = sum of weights)
  - evict: out_block = pv[:, 0:64] * (g/Z)           (ACT copy w/ scale AP)
  - blend: out += (1-g) * global_out                 (DVE)
The softmax normalization (1/Z), gate, and boundary masking are exact:
zero-padded K columns give exp(0)=1 which the band mask multiplies to 0,
and out-of-range V rows/ones-column entries are zero.
"""

import os
import sys
from contextlib import ExitStack

import numpy as np

sys.path.insert(0, "/opt/trn_rl_repo")

import concourse.bacc as bacc  # noqa: E402
import concourse.tile as tile  # noqa: E402
from concourse import mybir  # noqa: E402
from concourse.bass_utils import run_bass_kernel_spmd  # noqa: E402

B, H, S, D = 2, 16, 2048, 64
RADIUS = 16
NCORES = 8
PAIRS = B * H            # 32
PPC = PAIRS // NCORES    # 4 pairs per core
NB = S // 128            # 16 query/key blocks
NCH = NB + 1             # 17 key chunks (chunk c covers keys [c*128-16, c*128+112))
SPAN = 160               # query span per key chunk
QT_W = 32 + S + 128      # 2208 padded Q_T width (col i <-> query i-32)
KT_W = 16 + S + 112      # 2176 padded K_T width (col j <-> key j-16)
VS_W = NCH * 65          # V chunks with appended ones/invg column

F32 = mybir.dt.float32
BF16 = mybir.dt.bfloat16

import ml_dtypes  # noqa: E402

NP_BF16 = ml_dtypes.bfloat16

# set by test harness to capture an NTFF profile
TRACE = bool(int(os.environ.get("KERNEL_TRACE", "0")))
LAST_RESULT = None

_CACHE = {}


def _build_program(nc, reps=None):
    # qt/kt stack two pairs on the partition axis (pair 2j on partitions
    # 0:64, pair 2j+1 on 64:128) so their DMAs use all 16 ports.
    qt_d = nc.dram_tensor("qt", [PPC // 2, 128, QT_W], F32, kind="ExternalInput")
    kt_d = nc.dram_tensor("kt", [PPC // 2, 128, KT_W], F32, kind="ExternalInput")
    vs_d = nc.dram_tensor("vs", [PPC, 128, VS_W], BF16, kind="ExternalInput")
    gl_d = nc.dram_tensor("gl", [PPC, 128, NB * 64], F32, kind="ExternalInput")
    mask_d = nc.dram_tensor("mask", [128, SPAN], BF16, kind="ExternalInput")
    wg_d = nc.dram_tensor("wg", [128, 1], F32, kind="ExternalInput")
    bgn_d = nc.dram_tensor("bgn", [1, 1], F32, kind="ExternalInput")
    out_d = nc.dram_tensor("out", [PPC, 128, NB * 64], F32, kind="ExternalOutput")

    with tile.TileContext(nc) as tc, ExitStack() as ctx:
        consts = ctx.enter_context(tc.tile_pool(name="consts", bufs=1))
        pairp = ctx.enter_context(tc.tile_pool(name="pairp", bufs=3))
        smalls = ctx.enter_context(tc.tile_pool(name="smalls", bufs=2))
        ppool = ctx.enter_context(tc.tile_pool(name="ppool", bufs=4))
        zpool = ctx.enter_context(tc.tile_pool(name="zpool", bufs=4))
        ps_s = ctx.enter_context(tc.tile_pool(name="ps_s", bufs=3, space="PSUM"))
        ps_pv = ctx.enter_context(tc.tile_pool(name="ps_pv", bufs=4, space="PSUM"))
        ps_g = ctx.enter_context(tc.tile_pool(name="ps_g", bufs=1, space="PSUM"))

        mask_sb = consts.tile([128, SPAN], BF16, tag="mask")
        nc.sync.dma_start(out=mask_sb, in_=mask_d[:, :])
        wg_sb = consts.tile([128, 1], F32, tag="wg")
        nc.sync.dma_start(out=wg_sb, in_=wg_d[:, :])
        bgn_sb = consts.tile([1, 1], F32, tag="bgn")
        nc.sync.dma_start(out=bgn_sb, in_=bgn_d[:, :])
        ones_sb = consts.tile([1, 128], F32, tag="ones")
        nc.vector.memset(ones_sb, 1.0)

        def emit_pair(p, qt2, kt2):
            b0 = (p % 2) * 64
            qt = qt2[b0 : b0 + 64, :]
            kt = kt2[b0 : b0 + 64, :]
            vs = pairp.tile([128, VS_W], BF16, tag="vs")
            nc.sync.dma_start(out=vs, in_=vs_d[p])
            vs3 = vs.rearrange("p (c w) -> p c w", w=65)
            gl = pairp.tile([128, NB * 64], F32, tag="gl")
            nc.sync.dma_start(out=gl, in_=gl_d[p])
            outp = pairp.tile([128, NB * 64], F32, tag="outp")

            # ---- gate: g = sigmoid(mean_s(q) . w + b), via exp only so the
            # ACT engine never has to swap activation tables ----
            g_ps = ps_g.tile([1, 128], F32, tag="gps")
            for t in range(NB):
                nc.tensor.matmul(
                    g_ps,
                    lhsT=wg_sb[b0 : b0 + 64, :],
                    rhs=qt[:, 32 + t * 128 : 32 + (t + 1) * 128],
                    start=(t == 0),
                    stop=(t == NB - 1),
                )
            # scl2 cols: 0 = sum(q.w), 3 = g, 4 = 1-g, 5 = 1/g = 1+exp(-x)
            scl2 = smalls.tile([1, 6], F32, tag="scl2")
            nc.vector.reduce_sum(scl2[:, 0:1], g_ps, axis=mybir.AxisListType.X)
            nc.scalar.activation(
                scl2[:, 5:6],
                scl2[:, 0:1],
                mybir.ActivationFunctionType.Exp,
                bias=bgn_sb[0:1, 0:1],
                scale=-1.0 / S,
            )
            nc.vector.tensor_scalar(
                scl2[:, 5:6], scl2[:, 5:6], 1.0, None, op0=mybir.AluOpType.add
            )
            nc.vector.reciprocal(scl2[:, 3:4], scl2[:, 5:6])
            nc.vector.tensor_scalar(
                scl2[:, 4:5],
                scl2[:, 3:4],
                -1.0,
                1.0,
                op0=mybir.AluOpType.mult,
                op1=mybir.AluOpType.add,
            )
            # broadcast (1-g, 1/g) across 128 partitions via ones matmul
            bc_ps = ps_s.tile([128, 2], F32, tag="st")
            nc.tensor.matmul(bc_ps, lhsT=ones_sb, rhs=scl2[:, 4:6], start=True, stop=True)
            bc = smalls.tile([128, 2], F32, tag="bc")
            nc.vector.tensor_copy(bc, bc_ps)
            # scale the appended V column (1 for valid keys) by 1/g so the
            # Z-column of pv comes out as Z/g and the evict scale is g/Z.
            nc.vector.tensor_scalar_mul(vs3[:, :, 64:65], vs3[:, :, 64:65], bc[:, 1:2])

            # ---- banded attention over 17 key chunks ----
            pvs = [None] * NB
            for c in range(NCH):
                st = ps_s.tile([128, SPAN], F32, tag="st")
                nc.tensor.matmul(
                    st,
                    lhsT=kt[:, c * 128 : (c + 1) * 128],
                    rhs=qt[:, c * 128 : c * 128 + SPAN],
                    start=True,
                    stop=True,
                )
                pT = ppool.tile([128, SPAN], BF16, tag="pT")
                # exp(scores / sqrt(D)) -- the 1/8 score scale rides the
                # activation's scale input
                nc.scalar.activation(
                    pT, st, mybir.ActivationFunctionType.Exp, scale=1.0 / np.sqrt(D)
                )
                # band masking on the otherwise-idle GpSimd engine
                nc.gpsimd.tensor_mul(pT, pT, mask_sb)
                if c < NB:
                    pv = ps_pv.tile([128, 65], F32, tag="pv")
                    pvs[c] = pv
                    nc.tensor.matmul(
                        pv,
                        lhsT=pT[:, 32:SPAN],
                        rhs=vs3[:, c, :],
                        start=True,
                        stop=False,
                        skip_group_check=True,
                    )
                if c > 0:
                    b = c - 1
                    nc.tensor.matmul(
                        pvs[b][96:128, :],
                        lhsT=pT[:, 0:32],
                        rhs=vs3[:, c, :],
                        start=False,
                        stop=True,
                        skip_group_check=True,
                        tile_position=(0, 96),
                    )
                    zr = zpool.tile([128, 1], F32, tag="zr")
                    nc.vector.reciprocal(zr, pvs[b][:, 64:65])
                    nc.scalar.activation(
                        outp[:, b * 64 : (b + 1) * 64],
                        pvs[b][:, 0:64],
                        mybir.ActivationFunctionType.Copy,
                        bias=0.0,
                        scale=zr,
                    )

            # ---- blend with global path ----
            nc.vector.tensor_scalar_mul(gl, gl, bc[:, 0:1])
            nc.vector.tensor_add(outp, outp, gl)
            nc.sync.dma_start(out=out_d[p], in_=outp)

        def emit_all():
            for grp in range(PPC // 2):
                qt2 = pairp.tile([128, QT_W], F32, tag="qt")
                nc.sync.dma_start(out=qt2, in_=qt_d[grp])
                kt2 = pairp.tile([128, KT_W], F32, tag="kt")
                nc.sync.dma_start(out=kt2, in_=kt_d[grp])
                for sub in range(2):
                    emit_pair(grp * 2 + sub, qt2, kt2)

        if reps is None:
            emit_all()
        else:
            # benchmark variant: repeat the whole body in-NEFF so wall-clock
            # deltas between rep counts measure pure HW iteration time
            engs = [
                mybir.EngineType.PE,
                mybir.EngineType.Activation,
                mybir.EngineType.DVE,
                mybir.EngineType.Pool,
                mybir.EngineType.SP,
            ]
            with tc.For_i(0, reps, 1, hint_engines=engs):
                emit_all()


def _get_nc(reps=None):
    key = ("nc", reps)
    if key not in _CACHE:
        nc = bacc.Bacc("TRN2", target_bir_lowering=False)
        _build_program(nc, reps=reps)
        nc.compile()
        _CACHE[key] = nc
    return _CACHE[key]


def _band_mask():
    j = np.arange(128)[:, None]
    i = np.arange(SPAN)[None, :]
    return ((j <= i) & (j >= i - 32)).astype(NP_BF16)


def _prepare_in_maps(inputs):
    q = np.ascontiguousarray(np.asarray(inputs["q"], dtype=np.float32))
    k = np.ascontiguousarray(np.asarray(inputs["k"], dtype=np.float32))
    v = np.ascontiguousarray(np.asarray(inputs["v"], dtype=np.float32))
    g = np.ascontiguousarray(np.asarray(inputs["global_out"], dtype=np.float32))
    wg = np.asarray(inputs["w_gate"], dtype=np.float32).reshape(64, 1)
    wg = np.ascontiguousarray(np.concatenate([wg, wg], axis=0))  # [128,1]
    bgn = -np.asarray(inputs["b_gate"], dtype=np.float32).reshape(1, 1)

    qf = q.reshape(PAIRS, S, D)
    kf = k.reshape(PAIRS, S, D)
    vf = v.reshape(PAIRS, S, D)
    gf = g.reshape(PAIRS, S, D)

    # host-side layout marshalling (transpose/pad/shift only, no math);
    # qt/kt stack pair 2j on partitions 0:64 and pair 2j+1 on 64:128
    qt = np.zeros((PAIRS // 2, 128, QT_W), np.float32)
    qt[:, 0:64, 32 : 32 + S] = qf[0::2].transpose(0, 2, 1)
    qt[:, 64:128, 32 : 32 + S] = qf[1::2].transpose(0, 2, 1)
    kt = np.zeros((PAIRS // 2, 128, KT_W), np.float32)
    kt[:, 0:64, 16 : 16 + S] = kf[0::2].transpose(0, 2, 1)
    kt[:, 64:128, 16 : 16 + S] = kf[1::2].transpose(0, 2, 1)

    vs = np.zeros((PAIRS, NCH * 128, 65), NP_BF16)
    vs[:, 16 : 16 + S, 0:64] = vf
    vs[:, 16 : 16 + S, 64] = 1.0
    vs = (
        vs.reshape(PAIRS, NCH, 128, 65)
        .transpose(0, 2, 1, 3)
        .reshape(PAIRS, 128, VS_W)
    )
    vs = np.ascontiguousarray(vs)

    gl = np.ascontiguousarray(
        gf.reshape(PAIRS, NB, 128, 64).transpose(0, 2, 1, 3).reshape(PAIRS, 128, NB * 64)
    )
    mask = _band_mask()

    in_maps = []
    for core in range(NCORES):
        lo, hi = core * PPC, (core + 1) * PPC
        glo, ghi = core * (PPC // 2), (core + 1) * (PPC // 2)
        in_maps.append(
            {
                "qt": np.ascontiguousarray(qt[glo:ghi]),
                "kt": np.ascontiguousarray(kt[glo:ghi]),
                "vs": vs[lo:hi],
                "gl": gl[lo:hi],
                "mask": mask,
                "wg": wg,
                "bgn": bgn,
            }
        )
    return in_maps


def kernel(**inputs):
    global LAST_RESULT
    in_maps = _prepare_in_maps(inputs)
    nc = _get_nc()
    try:
        res = run_bass_kernel_spmd(
            nc, in_maps, core_ids=list(range(NCORES)), trace=TRACE
        )
    except ModuleNotFoundError:
        # NTFF profiling hook unavailable in this axon build
        res = run_bass_kernel_spmd(
            nc, in_maps, core_ids=list(range(NCORES)), trace=False
        )
    LAST_RESULT = res

    outs = np.stack([res.results[i]["out"] for i in range(NCORES)])  # [8,4,128,NB*64]
    out = (
        outs.reshape(PAIRS, 128, NB, 64)
        .transpose(0, 2, 1, 3)
        .reshape(B, H, S, D)
    )
    return np.ascontiguousarray(out)


def bench_hw_ns(inputs, reps_lo=16, reps_hi=2064, runs=5):
    """Estimate per-invocation HW time via in-NEFF repetition.

    Runs the same program with the body looped reps_lo and reps_hi times;
    the wall-clock delta divided by the rep delta isolates on-device time
    from compile/shipping/dispatch overhead.
    """
    import time

    in_maps = _prepare_in_maps(inputs)

    def run_variant(reps):
        nc = _get_nc(reps=reps)
        times = []
        for r in range(runs + 1):
            t0 = time.time()
            run_bass_kernel_spmd(nc, in_maps, core_ids=list(range(NCORES)))
            t1 = time.time()
            if r > 0:  # first run includes NEFF compile
                times.append(t1 - t0)
        return min(times)

    t_lo = run_variant(reps_lo)
    t_hi = run_variant(reps_hi)
    per_iter_ns = (t_hi - t_lo) / (reps_hi - reps_lo) * 1e9
    return per_iter_ns, t_lo, t_hi


if __name__ == "__main__":
    rng = np.random.default_rng(0)
    ins = {
        "q": rng.standard_normal((B, H, S, D), dtype=np.float32),
        "k": rng.standard_normal((B, H, S, D), dtype=np.float32),
        "v": rng.standard_normal((B, H, S, D), dtype=np.float32),
        "global_out": rng.standard_normal((B, H, S, D), dtype=np.float32),
        "buckets": rng.integers(0, 64, size=(B, S)),
        "w_gate": rng.standard_normal(64, dtype=np.float32) / 8.0,
        "b_gate": np.zeros(1, np.float32),
    }
    out = kernel(**ins)
    print("out", out.shape, out.dtype, float(np.abs(out).max()))
